# revision 4
# baseline (speedup 1.0000x reference)
"""PSANet 'distribute' gather kernel for Trainium2 (8 NeuronCores, SPMD).

Problem:
    x: (N=2, 16129=127*127, H=64, W=64) f32
    out[n, h*64+w, i, j] = x[n, (i-h+63)*127 + (j-w+63), h, w]

Sharding: over the h part of the output-channel dim (h*64+w): core k owns
h in [8k, 8k+8).  Per-core shard = x[:, (56-h0)*127 : (56-h0+71)*127,
h0:h0+8, :], laid out w-major on the host:

    xsw[w, n, hl, pl, q] = x[n, (pl+56-h0)*127 + q, h0+hl, w]

Device kernel = 16 direct HBM->HBM DMA copies (no SBUF, no compute
engines).  The whole gather is a single *linear* access pattern per
(n, hl), because with q innermost the source offset of output element
(w, i, j) is

    off(w,i,j) = base + w*(SW-1) + i*127 + j          (SW = w-stride)

i.e. the w-dependent column shift (j+63-w) folds into the w stride as a
"diagonal" AP.  Source descriptors are the 256B rows x[n, c, h, w0:w0+64]
actually needed (reads exactly the useful 16.8 MB/core, vs 33.3 MB for
full rows); destination descriptors are fully contiguous 16 KB blocks
os[n, ch, :, :].

Per-core DMA traffic: 16.8 MB gathered once, split across both HWDGE
queues (SP + ACT) as w-halves.
"""

import numpy as np

N, H, W = 2, 64, 64
Q = 2 * W - 1          # 127
PW = 71                # per-core p-window width (union over 8 h values)
HL = 8                 # h values per core
NCORES = 8

_cache = {}


def _build_bass(repeat=1, wsplit=2):
    import concourse.bass as bass
    import concourse.mybir as mybir
    from concourse.tile import TileContext

    def _split_multi_waits():
        """This container's walrus accepts at most ONE sync-wait per
        instruction; Tile's wait assignment can attach several.  Hoist
        extra waits onto NOPs inserted right before the instruction on
        the same engine (sequencers execute waits in program order, so
        semantics are identical)."""
        for fn in nc.m.functions:
            for blk in fn.blocks:
                old = blk.instructions
                new = []
                changed = False
                for inst in old:
                    si = inst.sync_info
                    waits = list(si.on_wait) if si is not None and si.on_wait else []
                    if len(waits) > 1:
                        changed = True
                        for wdesc in waits[:-1]:
                            nop = mybir.InstNoOp(
                                name=nc.get_next_instruction_name(), ins=[], outs=[]
                            )
                            nop.engine = inst.engine
                            nop.sync_info = mybir.SyncInfo(
                                on_wait=[wdesc], on_update=list()
                            )
                            new.append(nop)
                        si.on_wait = [waits[-1]]
                        inst.sync_info = si
                    new.append(inst)
                if changed:
                    blk.instructions = new

    f32 = mybir.dt.float32
    nc = bass.Bass(trn_type="TRN2")
    xsw = nc.dram_tensor("xsw", [W, N, HL, PW, Q], f32, kind="ExternalInput")
    os = nc.dram_tensor("os", [N, HL * W, H, W], f32, kind="ExternalOutput")

    SW = N * HL * PW * Q  # per-w pitch in elements (144272)
    WB = W // wsplit      # w-values per queue

    with TileContext(nc):
        for _rep in range(repeat):
            for n in range(N):
                for hl in range(HL):
                    base_in = n * HL * PW * Q + hl * PW * Q + (7 - hl) * Q + 63
                    base_out = (n * HL * W + hl * W) * H * W
                    for s in range(wsplit):
                        src = bass.AP(
                            tensor=xsw,
                            offset=base_in + s * WB * (SW - 1),
                            ap=[[SW - 1, WB], [Q, H], [1, W]],
                        )
                        dst = bass.AP(
                            tensor=os,
                            offset=base_out + s * WB * H * W,
                            ap=[[H * W, WB], [1, H * W]],
                        )
                        eng = nc.sync if s % 2 == 0 else nc.scalar
                        eng.dma_start(out=dst, in_=src)
    _split_multi_waits()
    return nc


def make_in_maps(x):
    """Per-core host shards: pure slice + dim transpose (no reindexing)."""
    in_maps = []
    for k in range(NCORES):
        h0 = HL * k
        c0 = (56 - h0) * Q
        xs = x[:, c0 : c0 + PW * Q, h0 : h0 + HL, :]
        xsw = np.ascontiguousarray(
            xs.reshape(N, PW, Q, HL, W).transpose(4, 0, 3, 1, 2)
        )
        in_maps.append({"xsw": xsw})
    return in_maps


def kernel(x):
    from concourse import bass_utils

    x = np.ascontiguousarray(np.asarray(x, dtype=np.float32))
    assert x.shape == (N, Q * Q, H, W), x.shape

    if "nc" not in _cache:
        _cache["nc"] = _build_bass()
    nc = _cache["nc"]

    in_maps = make_in_maps(x)
    res = bass_utils.run_bass_kernel_spmd(nc, in_maps, core_ids=list(range(NCORES)))
    out = np.concatenate([r["os"] for r in res.results], axis=1)
    return out
